# revision 22
# baseline (speedup 1.0000x reference)
"""Trainium2 Bass kernel for nn_ARIMAModel (depthwise causal conv, 8 taps).

Math: reference output = window_part(x, ar) + window_part(x, ma); both windows
have k == 8 and window_part is linear in the weights, so

    out[b,n,i,f] = sum_{a=0}^{7} C[a,f] * x[b,n,i-8+a,f]   (i >= 8, else 0)
    C = ar_params + ma_params

Flattening each (b,n) sequence to a stream of S*F elements, the conv is a
banded linear map on 256-element blocks of the stream:

    out[256C + u] = sum_lag C[8-lag, u%8] * xpad[256C + 128 + (u - 8*lag)]

i.e. three 128-contraction matmuls per 256-wide output block, with small
banded weight matrices built from C on the host.  Data-parallel over 8 cores
(100 sequences each); no cross-device communication.

Mode "pe" (default) per-core pipeline:
  - host: cast x to fp16 and build a (128, L+128) sliding-window view of the
    padded stream (partition p owns span [p*L, p*L+L) plus a 128-elem left
    halo; L = 25600).
  - plain big-descriptor DMA loads (6.4KB/partition chunks, ~350 GB/s);
    NO DMA-xbar transpose (the xbar serializes against every other DMA on
    the core and caps at ~240 GB/s, measured).
  - PE transpose-mode matmuls (~75ns/tile) build the X^T tiles on-chip in
    f16 PSUM quads; DVE/ACT copy them to SBUF.
  - conv: per 256-wide output group g, 3 matmuls contract the transposed
    tiles T_{2g}, T_{2g+1}, T_{2g+2} against banded weights [W0|Wm1]
    (the first matmul is N=256 with zero-padded weights so its start=True
    clears every PSUM has_written bit the group touches).
  - PSUM (fp32) -> SBUF copies cast to fp16 and stage 10 groups; output rows
    are span-major, so the store DMA writes 5.1KB-contiguous DRAM rows and
    overlaps the input loads (both plain DMA -> no xbar serialization).
  - host: reassemble spans, cast fp16->fp32, zero the first 8 stations of
    every sequence (conv warm-up region; also absorbs the cross-sequence
    contamination in the flat stream).

End-to-end absmax-relative error vs the fp32 reference: ~6e-4 (fp16 input /
weight / output quantization; accumulation is fp32 in PSUM).

Other modes kept for reference/fallback: "fp16" (DMA-xbar transpose input
path, phase-separated from the output stream; ~64us), "hybrid" (xbar + PE
transpose split; ~75us), "bf16_split" (x and C each split into bf16 hi+lo
parts -> ~8.5e-6 rel err at ~146us, fp32-grade accuracy fallback).
"""

import numpy as np
import ml_dtypes

BF16 = ml_dtypes.bfloat16

MODE = "pe"                             # "pe" | "hybrid" | "fp16" | "bf16_split"

B, N, S, F = 4, 200, 4096, 8
K = 8
NCORES = 8
P = 128
SEQ_PER_CORE = B * N // NCORES          # 100
STREAM = SEQ_PER_CORE * S * F           # 3,276,800 elements per core
NBLK = STREAM // P                      # 25,600 blocks of 128

# fp16-mode tiling
CB = 5120                               # 128-blocks per chunk
OT_BANKS = 5                            # PSUM banks staged per output DMA

# bf16_split-mode tiling
SP_CB = 5120
SP_GROUP = 4
SP_OT_GROUPS = 5

_compiled = {}


# --------------------------------------------------------------------------
# fp16 mode
# --------------------------------------------------------------------------

def _make_nc_fp16(nblk, cb, ot_banks, n_cores):
    import concourse.mybir as mybir
    import concourse.tile as tile
    from concourse import bacc

    chunks = nblk // cb
    assert chunks * cb == nblk
    tw = cb + P                         # transposed cols per chunk (halo incl.)
    tw2 = tw // 2
    ncoarse = nblk // 2                 # 256-elem output blocks per core
    subtiles_per_chunk = cb // 256      # psum half-bank groups of 128 coarse
    banks_per_chunk = subtiles_per_chunk // 2
    otiles_per_chunk = banks_per_chunk // ot_banks
    assert otiles_per_chunk * ot_banks == banks_per_chunk
    ot_cols = ot_banks * 512            # output cols per staging tile

    nc = bacc.Bacc(
        "TRN2", target_bir_lowering=False, debug=False, num_devices=n_cores
    )
    f16 = mybir.dt.float16
    f32 = mybir.dt.float32

    # chunked + parity-deinterleaved input: x_d[c, j, :] rows are the chunk's
    # even 128-blocks then its odd 128-blocks (host lays this out)
    x_d = nc.dram_tensor("x16", [chunks, tw, P], f16, kind="ExternalInput")
    # weights: [W0 (256 cols, zero-padded) | Wm1 (64 cols)], stored
    # TRANSPOSED on host so the load can use the xbar-transpose path (keeps
    # phase 1 free of DMA-mode transitions)
    w_d = nc.dram_tensor("wts", [320, P], f16, kind="ExternalInput")
    y_d = nc.dram_tensor("y", [ncoarse, 256], f16, kind="ExternalOutput")

    def _ins(x):
        return getattr(x, "ins", x)

    with tile.TileContext(nc) as tc:
        from concourse.tile import add_dep_helper
        with tc.tile_pool(name="wpool", bufs=1) as wpool, \
             tc.tile_pool(name="xpool", bufs=chunks) as xpool, \
             tc.tile_pool(name="psum", bufs=8, space="PSUM") as psum, \
             tc.tile_pool(name="opool", bufs=chunks * otiles_per_chunk) as opool:
            W = wpool.tile([P, 320], f16)
            nc.sync.dma_start(out=W[:], in_=w_d[:], transpose=True)
            # Phase 1: all xbar transposes (SP ring), with PE matmuls and
            # PSUM->SBUF copies overlapping as chunks land.  Phase 2: output
            # DMAs, explicitly held until the LAST transpose completes -- the
            # HW xbar-mode bug forces Tile to serialize any transpose/copy
            # DMA pair, so interleaving them thrashes; one transition is free.
            tr_insts = []
            out_calls = []
            copy_flip = 0
            for c in range(chunks):
                xt = xpool.tile([P, tw], f16, tag="xt")
                tr = nc.sync.dma_start(out=xt[:], in_=x_d[c], transpose=True)
                tr_insts.append(_ins(tr))
                for ot in range(otiles_per_chunk):
                    otile = opool.tile([P, ot_cols], f16)
                    for g in range(ot_banks):
                        pt = psum.tile([P, 512], f32)
                        for half in range(2):
                            i = (ot * ot_banks + g) * 2 + half
                            A = i * P
                            o0 = half * 256
                            # S0 = odd blocks, S1/Sm1 = even blocks
                            s0 = xt[:, tw2 + A: tw2 + A + P]
                            s1 = xt[:, A + 1: A + 1 + P]
                            sm1 = xt[:, A: A + P]
                            nc.tensor.matmul(pt[:, o0: o0 + 256], s0,
                                             W[:, 0:256],
                                             start=True, stop=False)
                            nc.tensor.matmul(pt[:, o0 + 128: o0 + 256], s1,
                                             W[:, 0:128],
                                             start=False, stop=False)
                            nc.tensor.matmul(pt[:, o0: o0 + 64], sm1,
                                             W[:, 256:320],
                                             start=False, stop=True)
                        odst = otile[:, g * 512:(g + 1) * 512]
                        if copy_flip % 2 == 0:
                            nc.vector.tensor_copy(odst, pt[:])
                        else:
                            nc.scalar.copy(odst, pt[:])
                        copy_flip += 1
                    base = (c * banks_per_chunk + ot * ot_banks) * 256
                    out = nc.scalar.dma_start(
                        out=y_d[base: base + ot_banks * 256, :].rearrange(
                            "(m p) u -> p m u", p=P
                        ),
                        in_=otile[:].rearrange("p (m u) -> p m u", u=256),
                    )
                    out_calls.append(_ins(out))
            for o in out_calls:
                add_dep_helper(o, tr_insts[-1],
                               reason="hold output DMAs until last transpose")
    nc.compile()
    return nc


def _build_wts_fp16(Cmat, transposed=True):
    """[W0(256, zero-padded) | Wm1(64)] from C (8x8 fp32), in fp16.

    out[256C+u] = sum_lag C[8-lag, u%8] * xpad[256C+128 + (u-8*lag)]
      S0[v]  = xpad[256C+128+v]  -> W0[v, v+8lag]            (u = v+8lag)
      S1[v]  = xpad[256C+256+v]  -> W0[v, v+8lag] cols <128  (u = 128+v+8lag)
      Sm1[v] = xpad[256C+v]      -> Wm1[v, v-128+8lag]       (u = v-128+8lag)
    """
    C16 = Cmat.astype(np.float16).astype(np.float32)
    W0 = np.zeros((P, 256), np.float32)
    Wm1 = np.zeros((P, 64), np.float32)
    for v in range(P):
        f = v % 8
        for lag in range(1, 9):
            u = v + 8 * lag
            if u < 256:
                W0[v, u] = C16[8 - lag, f]
            um = v - 128 + 8 * lag
            if 0 <= um < 64:
                Wm1[v, um] = C16[8 - lag, f]
    W = np.concatenate([W0, Wm1], axis=1)
    if transposed:
        W = np.ascontiguousarray(W.T)
    return W.astype(np.float16)


def _prep_in_maps_fp16(x, ar_params, ma_params, n_cores, stream, nblk, cb):
    chunks = nblk // cb
    tw = cb + P
    padded = nblk + P
    Cmat = np.asarray(ar_params, np.float32) + np.asarray(ma_params, np.float32)
    wts = _build_wts_fp16(Cmat)
    xf = np.ascontiguousarray(np.asarray(x, dtype=np.float32)).reshape(
        n_cores, stream
    )
    pad = np.zeros((n_cores, padded, P), np.float16)
    pad[:, 1:1 + nblk, :] = xf.astype(np.float16).reshape(n_cores, nblk, P)
    # per-chunk parity de-interleave: even blocks then odd blocks
    perm = np.concatenate([np.arange(0, tw, 2), np.arange(1, tw, 2)])
    xd = np.empty((n_cores, chunks, tw, P), np.float16)
    for c in range(chunks):
        xd[:, c] = pad[:, c * cb: c * cb + tw, :][:, perm, :]
    return [
        {"x16": xd[core], "wts": wts} for core in range(n_cores)
    ]


# --------------------------------------------------------------------------
# bf16_split mode (fp32-grade fallback)
# --------------------------------------------------------------------------

def _make_nc_split(nblk, cb, group, ot_groups, n_cores):
    import concourse.mybir as mybir
    import concourse.tile as tile
    from concourse import bacc

    chunks = nblk // cb
    assert chunks * cb == nblk
    tw = cb + P
    tiles_per_chunk = cb // P
    groups_per_chunk = tiles_per_chunk // group
    otiles_per_chunk = groups_per_chunk // ot_groups
    assert otiles_per_chunk * ot_groups == groups_per_chunk
    ot_cols = ot_groups * group * P

    nc = bacc.Bacc(
        "TRN2", target_bir_lowering=False, debug=False, num_devices=n_cores
    )
    bf = mybir.dt.bfloat16
    f32 = mybir.dt.float32

    xh_d = nc.dram_tensor("xh", [nblk + P, P], bf, kind="ExternalInput")
    xl_d = nc.dram_tensor("xl", [nblk + P, P], bf, kind="ExternalInput")
    w_d = nc.dram_tensor("wts", [4, P, P], bf, kind="ExternalInput")
    y_d = nc.dram_tensor("y", [nblk, P], f32, kind="ExternalOutput")

    with tile.TileContext(nc) as tc:
        with tc.tile_pool(name="wpool", bufs=1) as wpool, \
             tc.tile_pool(name="xpool", bufs=2) as xpool, \
             tc.tile_pool(name="psum", bufs=6, space="PSUM") as psum, \
             tc.tile_pool(name="opool", bufs=2) as opool:
            W = wpool.tile([P, 4, P], bf)
            for j in range(4):
                nc.sync.dma_start(out=W[:, j, :], in_=w_d[j])
            for c in range(chunks):
                xh_t = xpool.tile([P, tw], bf, tag="xh_t")
                xl_t = xpool.tile([P, tw], bf, tag="xl_t")
                nc.sync.dma_start(
                    out=xh_t[:], in_=xh_d[c * cb: c * cb + tw, :],
                    transpose=True
                )
                nc.sync.dma_start(
                    out=xl_t[:], in_=xl_d[c * cb: c * cb + tw, :],
                    transpose=True
                )
                for ot in range(otiles_per_chunk):
                    otile = opool.tile([P, ot_cols], f16)
                    for g in range(ot_groups):
                        pt = psum.tile([P, group * P], f32)
                        for k in range(group):
                            t = (ot * ot_groups + g) * group + k
                            s = t * P
                            o = pt[:, k * P:(k + 1) * P]
                            nc.tensor.matmul(o, xh_t[:, s + 1: s + 1 + P],
                                             W[:, 0, :], start=True, stop=False)
                            nc.tensor.matmul(o, xh_t[:, s + 1: s + 1 + P],
                                             W[:, 2, :], start=False, stop=False)
                            nc.tensor.matmul(o, xh_t[:, s: s + P],
                                             W[:, 1, :], start=False, stop=False)
                            nc.tensor.matmul(o, xh_t[:, s: s + P],
                                             W[:, 3, :], start=False, stop=False)
                            nc.tensor.matmul(o, xl_t[:, s + 1: s + 1 + P],
                                             W[:, 0, :], start=False, stop=False)
                            nc.tensor.matmul(o, xl_t[:, s: s + P],
                                             W[:, 1, :], start=False, stop=True)
                        odst = otile[:, g * group * P:(g + 1) * group * P]
                        if g % 2 == 0:
                            nc.vector.tensor_copy(odst, pt[:])
                        else:
                            nc.scalar.copy(odst, pt[:])
                    base = c * cb + ot * ot_cols
                    nc.scalar.dma_start(
                        out=y_d[base: base + ot_cols, :].rearrange(
                            "(k p) u -> p k u", p=P
                        ),
                        in_=otile[:].rearrange("p (k u) -> p k u", u=P),
                    )
    nc.compile()
    return nc


def _mk_AB(Cm):
    A = np.zeros((P, P), np.float32)
    Bm = np.zeros((P, P), np.float32)
    for u in range(P):
        f = u % 8
        for lag in range(1, 9):
            coef = Cm[8 - lag, f]
            v = u - 8 * lag
            if v >= 0:
                A[v, u] = coef
            else:
                Bm[v + P, u] = coef
    return A, Bm


def _build_wts_split(Cmat):
    Chi = Cmat.astype(BF16).astype(np.float32)
    Clo = (Cmat - Chi).astype(BF16).astype(np.float32)
    Ahi, Bhi = _mk_AB(Chi)
    Alo, Blo = _mk_AB(Clo)
    return np.stack([Ahi, Bhi, Alo, Blo]).astype(BF16)


def _prep_in_maps_split(x, ar_params, ma_params, n_cores, stream, nblk):
    padded = nblk + P
    Cmat = np.asarray(ar_params, np.float32) + np.asarray(ma_params, np.float32)
    wts = _build_wts_split(Cmat)
    xf = np.ascontiguousarray(np.asarray(x, dtype=np.float32)).reshape(
        n_cores, stream
    )
    xh = xf.astype(BF16)
    xl = (xf - xh.astype(np.float32)).astype(BF16)
    ph = np.zeros((n_cores, padded * P), BF16)
    ph[:, P:P + stream] = xh
    pl = np.zeros((n_cores, padded * P), BF16)
    pl[:, P:P + stream] = xl
    return [
        {
            "xh": ph[c].reshape(padded, P),
            "xl": pl[c].reshape(padded, P),
            "wts": wts,
        }
        for c in range(n_cores)
    ]


# --------------------------------------------------------------------------
# pe mode: no DMA-xbar at all.  Plain big-descriptor loads (overlap the
# output stream freely), PE transpose-mode matmuls build the X^T tiles
# on-chip, and the span-major layout makes output rows ~10KB contiguous.
# --------------------------------------------------------------------------

def _make_nc_pe(L, load_cols, g_stage, n_cores):
    import concourse.mybir as mybir
    import concourse.tile as tile
    from concourse import bacc

    NJ = L // P + 1                      # 128-col transpose tiles (incl halo)
    NG = L // 256                        # 256-elem output groups per partition
    # variable load plan: small first loads so the PE pipeline ramps early
    jtot = NJ - 1
    load_plan = []
    for want in (4, 8, 12):
        if jtot > want:
            load_plan.append(want)
            jtot -= want
    while jtot > 0:
        take = min(load_cols // P, jtot)
        load_plan.append(take)
        jtot -= take
    assert sum(load_plan) == NJ - 1
    assert NG % g_stage == 0
    notiles = NG // g_stage

    nc = bacc.Bacc(
        "TRN2", target_bir_lowering=False, debug=False, num_devices=n_cores
    )
    f16 = mybir.dt.float16
    f32 = mybir.dt.float32

    x_d = nc.dram_tensor("xin", [P, L + P], f16, kind="ExternalInput")
    w_d = nc.dram_tensor("wts", [P, 320], f16, kind="ExternalInput")
    id_d = nc.dram_tensor("ident", [P, P], f16, kind="ExternalInput")
    y_d = nc.dram_tensor("y", [P, L], f16, kind="ExternalOutput")

    with tile.TileContext(nc) as tc:
        with tc.tile_pool(name="wpool", bufs=1) as wpool, \
             tc.tile_pool(name="xpool", bufs=len(load_plan) + 1) as xpool, \
             tc.tile_pool(name="tq", bufs=4) as tqpool, \
             tc.tile_pool(name="pst", bufs=3, space="PSUM") as pst, \
             tc.tile_pool(name="pso", bufs=4, space="PSUM") as pso, \
             tc.tile_pool(name="opool", bufs=2) as opool:
            W = wpool.tile([P, 320], f16, tag="w")
            ident = wpool.tile([P, P], f16, tag="ident")
            nc.sync.dma_start(out=W[:], in_=w_d[:])
            nc.sync.dma_start(out=ident[:], in_=id_d[:])
            xts = []          # (tile, j_base, n_j)
            jb = 0
            for nj in load_plan:
                xt = xpool.tile([P, nj * P], f16, tag=f"xin{min(nj,32)}")
                nc.sync.dma_start(
                    out=xt[:], in_=x_d[:, jb * P:(jb + nj) * P]
                )
                xts.append((xt, jb, nj))
                jb += nj
            xhalo = xpool.tile([P, P], f16, tag="xhalo")
            nc.sync.dma_start(out=xhalo[:], in_=x_d[:, L:])

            def src_of(j):
                if j == NJ - 1:
                    return xhalo[:, 0:P]
                for xt, jb2, nj in xts:
                    if jb2 <= j < jb2 + nj:
                        return xt[:, (j - jb2) * P:(j - jb2 + 1) * P]
                raise AssertionError(j)

            # T_j (transposed tiles) are built in quads: 4 PE transposes into
            # one f16 PSUM bank, one copy out to SBUF.
            tq_tiles = {}                # quad index -> sbuf tile
            def t_of(j):
                q, off = j // 4, (j % 4) * P
                return tq_tiles[q][:, off: off + P]

            nquads = (NJ + 3) // 4
            copy_flip = 0
            g_next = 0
            otile = None
            for q in range(nquads):
                ptile = pst.tile([P, 512], f16)
                j_hi = min(4 * q + 4, NJ)
                for j in range(4 * q, j_hi):
                    nc.tensor.transpose(
                        ptile[:, (j % 4) * P:(j % 4 + 1) * P], src_of(j),
                        ident[:]
                    )
                tqt = tqpool.tile([P, 512], f16, tag="tq")
                if q % 2 == 0:
                    nc.vector.tensor_copy(tqt[:], ptile[:])
                else:
                    nc.scalar.copy(tqt[:], ptile[:])
                tq_tiles[q] = tqt
                # emit conv groups whose inputs are now all transposed
                while g_next < NG and 2 * g_next + 2 < j_hi:
                    g = g_next
                    if g % 2 == 0:
                        po = pso.tile([P, 512], f32)
                    o0 = (g % 2) * 256
                    nc.tensor.matmul(po[:, o0: o0 + 256], t_of(2 * g + 1),
                                     W[:, 0:256], start=True, stop=False)
                    nc.tensor.matmul(po[:, o0 + 128: o0 + 256], t_of(2 * g + 2),
                                     W[:, 0:128], start=False, stop=False)
                    nc.tensor.matmul(po[:, o0: o0 + 64], t_of(2 * g),
                                     W[:, 256:320], start=False, stop=True)
                    if g % 2 == 1:
                        if g // 2 % (g_stage // 2) == 0:
                            otile = opool.tile([P, g_stage * 256], f16,
                                               tag="ot")
                        oc = (g // 2 % (g_stage // 2)) * 512
                        odst = otile[:, oc: oc + 512]
                        if copy_flip % 2 == 0:
                            nc.vector.tensor_copy(odst, po[:])
                        else:
                            nc.scalar.copy(odst, po[:])
                        copy_flip += 1
                        if (g + 1) % g_stage == 0:
                            o_idx = g // g_stage
                            nc.scalar.dma_start(
                                out=y_d[:, o_idx * g_stage * 256:
                                        (o_idx + 1) * g_stage * 256],
                                in_=otile[:],
                            )
                    g_next += 1
    nc.compile()
    return nc


def _prep_in_maps_pe(x, ar_params, ma_params, n_cores, stream, L):
    Cmat = np.asarray(ar_params, np.float32) + np.asarray(ma_params, np.float32)
    wts = _build_wts_fp16(Cmat, transposed=False)
    xf = np.ascontiguousarray(np.asarray(x, dtype=np.float32)).reshape(
        n_cores, stream
    )
    xpad = np.zeros((n_cores, P + stream), np.float16)
    xpad[:, P:] = xf.astype(np.float16)
    ident = np.eye(P, dtype=np.float16)
    maps = []
    for c in range(n_cores):
        win = np.lib.stride_tricks.as_strided(
            xpad[c], (P, L + P), (L * 2, 2)
        )
        maps.append({
            "xin": np.ascontiguousarray(win),
            "wts": wts,
            "ident": ident,
        })
    return maps


# --------------------------------------------------------------------------
# hybrid mode: stream split in two.  Part A goes through the DMA-xbar
# transpose path in an exclusive phase (the xbar serializes against every
# other DMA, so nothing else moves while it runs -- but the PE computes A's
# convolutions underneath it).  Part B uses plain big-descriptor loads +
# PE transpose-mode, and all output stores run in phase B where they overlap
# the B loads.  W / identity are ALSO loaded via the xbar so phase A contains
# no DMA mode transitions at all.
# --------------------------------------------------------------------------

def _make_nc_hybrid(nblkA, cbA, ot_banksA, L_B, load_colsB, g_stageB, n_cores):
    import concourse.mybir as mybir
    import concourse.tile as tile
    from concourse import bacc
    from concourse.tile import add_dep_helper

    # ---- A-side geometry (xbar path, fp16-mode structure)
    chunksA = nblkA // cbA
    assert chunksA * cbA == nblkA
    twA = cbA + P
    twA2 = twA // 2
    ncoarseA = nblkA // 2
    subtilesA = cbA // 256
    banksA = subtilesA // 2
    otilesA = banksA // ot_banksA
    assert otilesA * ot_banksA == banksA
    ot_colsA = ot_banksA * 512

    # ---- B-side geometry (pe path)
    NJ = L_B // P + 1
    NG = L_B // 256
    jgrp = load_colsB // P
    assert (NJ - 1) % jgrp == 0
    nloadsB = (NJ - 1) // jgrp
    assert NG % g_stageB == 0

    nc = bacc.Bacc(
        "TRN2", target_bir_lowering=False, debug=False, num_devices=n_cores
    )
    f16 = mybir.dt.float16
    f32 = mybir.dt.float32

    xA_d = nc.dram_tensor("xA", [chunksA, twA, P], f16, kind="ExternalInput")
    xB_d = nc.dram_tensor("xB", [P, L_B + P], f16, kind="ExternalInput")
    w_d = nc.dram_tensor("wts", [320, P], f16, kind="ExternalInput")
    id_d = nc.dram_tensor("ident", [P, P], f16, kind="ExternalInput")
    yA_d = nc.dram_tensor("yA", [ncoarseA, 256], f16, kind="ExternalOutput")
    yB_d = nc.dram_tensor("yB", [P, L_B], f16, kind="ExternalOutput")

    def _ins(x):
        return getattr(x, "ins", x)

    plain_dmas = []
    early_loads = []
    with tile.TileContext(nc) as tc:
        with tc.tile_pool(name="wpool", bufs=1) as wpool, \
             tc.tile_pool(name="xpoolA", bufs=chunksA) as xpoolA, \
             tc.tile_pool(name="xpoolB", bufs=nloadsB + 1) as xpoolB, \
             tc.tile_pool(name="tq", bufs=4) as tqpool, \
             tc.tile_pool(name="psA", bufs=3, space="PSUM") as psA, \
             tc.tile_pool(name="pst", bufs=2, space="PSUM") as pst, \
             tc.tile_pool(name="psB", bufs=3, space="PSUM") as psB, \
             tc.tile_pool(name="opoolA", bufs=otilesA * chunksA) as opoolA, \
             tc.tile_pool(name="opoolB", bufs=NG // g_stageB) as opoolB:
            W = wpool.tile([P, 320], f16, tag="w")
            ident = wpool.tile([P, P], f16, tag="ident")

            # phase 0: ALL plain input loads (B spans), before any xbar use
            xts = []
            for gl in range(nloadsB):
                xbt = xpoolB.tile([P, load_colsB], f16, tag="xinB")
                ld = nc.sync.dma_start(
                    out=xbt[:],
                    in_=xB_d[:, gl * load_colsB:(gl + 1) * load_colsB],
                )
                early_loads.append(_ins(ld))
                xts.append(xbt)
            xhalo = xpoolB.tile([P, P], f16, tag="xhaloB")
            ldh = nc.sync.dma_start(out=xhalo[:], in_=xB_d[:, L_B:])
            early_loads.append(_ins(ldh))

            # phase X: xbar transposes (W, ident, A chunks); PE does B work
            wtr = nc.sync.dma_start(out=W[:], in_=w_d[:], transpose=True)
            itr = nc.sync.dma_start(out=ident[:], in_=id_d[:], transpose=True)
            tr_insts = [_ins(wtr), _ins(itr)]
            xtAs = []
            for c in range(chunksA):
                xtA = xpoolA.tile([P, twA], f16, tag="xtA")
                tr = nc.sync.dma_start(out=xtA[:], in_=xA_d[c], transpose=True)
                tr_insts.append(_ins(tr))
                xtAs.append(xtA)
            # xbar only after the plain loads have fully drained
            for t in tr_insts:
                for el in early_loads:
                    add_dep_helper(t, el, sync=True,
                                   reason="xbar waits for plain input loads")

            copy_flip = 0

            # ---- B section: PE transposes + convs (data from phase 0)
            tq_tiles = {}

            def t_of(j):
                q, off = j // 4, (j % 4) * P
                return tq_tiles[q][:, off: off + P]

            def src_of(j):
                if j == NJ - 1:
                    return xhalo[:, 0:P]
                return xts[j // jgrp][:, (j % jgrp) * P:(j % jgrp + 1) * P]

            nquads = (NJ + 3) // 4
            g_next = 0
            otile = None
            for q in range(nquads):
                ptile = pst.tile([P, 512], f16)
                j_hi = min(4 * q + 4, NJ)
                for j in range(4 * q, j_hi):
                    nc.tensor.transpose(
                        ptile[:, (j % 4) * P:(j % 4 + 1) * P], src_of(j),
                        ident[:]
                    )
                tqt = tqpool.tile([P, 512], f16, tag="tq")
                if q % 2 == 0:
                    nc.vector.tensor_copy(tqt[:], ptile[:])
                else:
                    nc.scalar.copy(tqt[:], ptile[:])
                tq_tiles[q] = tqt
                while g_next < NG and 2 * g_next + 2 < j_hi:
                    g = g_next
                    if g % 2 == 0:
                        po = psB.tile([P, 512], f32)
                    o0 = (g % 2) * 256
                    nc.tensor.matmul(po[:, o0: o0 + 256], t_of(2 * g + 1),
                                     W[:, 0:256], start=True, stop=False)
                    nc.tensor.matmul(po[:, o0 + 128: o0 + 256], t_of(2 * g + 2),
                                     W[:, 0:128], start=False, stop=False)
                    nc.tensor.matmul(po[:, o0: o0 + 64], t_of(2 * g),
                                     W[:, 256:320], start=False, stop=True)
                    if g % 2 == 1:
                        if g // 2 % (g_stageB // 2) == 0:
                            otile = opoolB.tile([P, g_stageB * 256], f16,
                                                tag="otB")
                        oc = (g // 2 % (g_stageB // 2)) * 512
                        odst = otile[:, oc: oc + 512]
                        if copy_flip % 2 == 0:
                            nc.vector.tensor_copy(odst, po[:])
                        else:
                            nc.scalar.copy(odst, po[:])
                        copy_flip += 1
                        if (g + 1) % g_stageB == 0:
                            o_idx = g // g_stageB
                            outb = nc.scalar.dma_start(
                                out=yB_d[:, o_idx * g_stageB * 256:
                                         (o_idx + 1) * g_stageB * 256],
                                in_=otile[:],
                            )
                            plain_dmas.append(_ins(outb))
                    g_next += 1

            # ---- A section: convs on xbar-transposed tiles
            for c in range(chunksA):
                xtA = xtAs[c]
                for ot in range(otilesA):
                    otileA = opoolA.tile([P, ot_colsA], f16, tag="otA")
                    for g in range(ot_banksA):
                        pt = psA.tile([P, 512], f32)
                        for half in range(2):
                            i = (ot * ot_banksA + g) * 2 + half
                            A0 = i * P
                            o0 = half * 256
                            s0 = xtA[:, twA2 + A0: twA2 + A0 + P]
                            s1 = xtA[:, A0 + 1: A0 + 1 + P]
                            sm1 = xtA[:, A0: A0 + P]
                            nc.tensor.matmul(pt[:, o0: o0 + 256], s0,
                                             W[:, 0:256],
                                             start=True, stop=False)
                            nc.tensor.matmul(pt[:, o0 + 128: o0 + 256], s1,
                                             W[:, 0:128],
                                             start=False, stop=False)
                            nc.tensor.matmul(pt[:, o0: o0 + 64], sm1,
                                             W[:, 256:320],
                                             start=False, stop=True)
                        odst = otileA[:, g * 512:(g + 1) * 512]
                        if copy_flip % 2 == 0:
                            nc.vector.tensor_copy(odst, pt[:])
                        else:
                            nc.scalar.copy(odst, pt[:])
                        copy_flip += 1
                    base = (c * banksA + ot * ot_banksA) * 256
                    outa = nc.scalar.dma_start(
                        out=yA_d[base: base + ot_banksA * 256, :].rearrange(
                            "(m p) u -> p m u", p=P
                        ),
                        in_=otileA[:].rearrange("p (m u) -> p m u", u=256),
                    )
                    plain_dmas.append(_ins(outa))

            for pd in plain_dmas:
                add_dep_helper(pd, tr_insts[-1],
                               reason="hold plain DMAs until last xbar transpose")
    nc.compile()
    return nc


def _prep_in_maps_hybrid(x, ar_params, ma_params, n_cores, stream,
                         nblkA, cbA, L_B):
    streamA = nblkA * P
    chunksA = nblkA // cbA
    twA = cbA + P
    paddedA = nblkA + P
    Cmat = np.asarray(ar_params, np.float32) + np.asarray(ma_params, np.float32)
    wts = _build_wts_fp16(Cmat, transposed=True)
    ident = np.ascontiguousarray(np.eye(P, dtype=np.float16))
    xf = np.ascontiguousarray(np.asarray(x, dtype=np.float32)).reshape(
        n_cores, stream
    )
    x16 = xf.astype(np.float16)
    # full padded stream (front 128 zeros) once per core
    xpadF = np.zeros((n_cores, P + stream), np.float16)
    xpadF[:, P:] = x16
    # A: chunked + parity-deinterleaved view of padded blocks [0, nblkA+P)
    padA = np.zeros((n_cores, paddedA, P), np.float16)
    padA.reshape(n_cores, -1)[:, :streamA + P] = xpadF[:, :streamA + P]
    perm = np.concatenate([np.arange(0, twA, 2), np.arange(1, twA, 2)])
    xA = np.empty((n_cores, chunksA, twA, P), np.float16)
    for c in range(chunksA):
        xA[:, c] = padA[:, c * cbA: c * cbA + twA, :][:, perm, :]
    maps = []
    for core in range(n_cores):
        winB = np.lib.stride_tricks.as_strided(
            xpadF[core, streamA:], (P, L_B + P), (L_B * 2, 2)
        )
        maps.append({
            "xA": xA[core],
            "xB": np.ascontiguousarray(winB),
            "wts": wts,
            "ident": ident,
        })
    return maps


# --------------------------------------------------------------------------
# driver
# --------------------------------------------------------------------------

HY_NBLKA = 12800
HY_CBA = 2560
HY_OTBA = 5
HY_LB = 12800
HY_LOADB = 3200
HY_GSTB = 10


def _get_nc(mode=MODE, **kw):
    if mode == "hybrid":
        key = ("hybrid", HY_NBLKA, HY_CBA, HY_OTBA, HY_LB, HY_LOADB, HY_GSTB,
               kw.get("n_cores", NCORES))
        if key not in _compiled:
            _compiled[key] = _make_nc_hybrid(*key[1:])
        return _compiled[key]
    if mode == "pe":
        key = ("pe", kw.get("L", STREAM // P), kw.get("load_cols", 3200),
               kw.get("g_stage", 20), kw.get("n_cores", NCORES))
        if key not in _compiled:
            _compiled[key] = _make_nc_pe(*key[1:])
        return _compiled[key]
    if mode == "fp16":
        key = ("fp16", kw.get("nblk", NBLK), kw.get("cb", CB),
               kw.get("ot_banks", OT_BANKS), kw.get("n_cores", NCORES))
        if key not in _compiled:
            _compiled[key] = _make_nc_fp16(*key[1:])
    else:
        key = ("split", kw.get("nblk", NBLK), kw.get("cb", SP_CB),
               kw.get("group", SP_GROUP), kw.get("ot_groups", SP_OT_GROUPS),
               kw.get("n_cores", NCORES))
        if key not in _compiled:
            _compiled[key] = _make_nc_split(*key[1:])
    return _compiled[key]


def _ensure_hook_shim():
    """run_bass_kernel_spmd(trace=True) imports antenv.axon_hooks, which the
    agent image may lack; also BASS_TRACE in the env triggers that path.
    Install a null shim so the import never crashes the kernel."""
    import sys
    import types
    try:
        import antenv.axon_hooks  # noqa: F401
    except Exception:
        mod = types.ModuleType("antenv.axon_hooks")
        mod.get_axon_ntff_profile_hook = lambda: None
        mod.set_axon_ntff_profile_hook = lambda h: None
        sys.modules["antenv.axon_hooks"] = mod


def _run(x, ar_params, ma_params, trace=False, mode=MODE, **run_kwargs):
    _ensure_hook_shim()
    from concourse.bass_utils import run_bass_kernel_spmd

    nc = _get_nc(mode)
    if mode == "hybrid":
        in_maps = _prep_in_maps_hybrid(x, ar_params, ma_params, NCORES, STREAM,
                                       HY_NBLKA, HY_CBA, HY_LB)
    elif mode == "pe":
        in_maps = _prep_in_maps_pe(x, ar_params, ma_params, NCORES, STREAM,
                                   STREAM // P)
    elif mode == "fp16":
        in_maps = _prep_in_maps_fp16(x, ar_params, ma_params, NCORES, STREAM,
                                     NBLK, CB)
    else:
        in_maps = _prep_in_maps_split(x, ar_params, ma_params, NCORES, STREAM,
                                      NBLK)
    res = run_bass_kernel_spmd(
        nc, in_maps, core_ids=list(range(NCORES)), trace=trace, **run_kwargs
    )
    if mode == "hybrid":
        out = np.empty((NCORES, STREAM), np.float32)
        sa = HY_NBLKA * P
        for c in range(NCORES):
            out[c, :sa] = np.asarray(
                res.results[c]["yA"], dtype=np.float32).reshape(-1)
            out[c, sa:] = np.asarray(
                res.results[c]["yB"], dtype=np.float32).reshape(-1)
    else:
        out = np.stack(
            [np.asarray(res.results[c]["y"], dtype=np.float32)
             for c in range(NCORES)]
        )
    out = out.reshape(B, N, S, F)
    out[:, :, :K, :] = 0.0
    return out, res


def kernel(x, ar_params, ma_params):
    out, _ = _run(x, ar_params, ma_params)
    return out


# revision 23
# speedup vs baseline: 1.0469x; 1.0469x over previous
"""Trainium2 Bass kernel for nn_ARIMAModel (depthwise causal conv, 8 taps).

Math: reference output = window_part(x, ar) + window_part(x, ma); both windows
have k == 8 and window_part is linear in the weights, so

    out[b,n,i,f] = sum_{a=0}^{7} C[a,f] * x[b,n,i-8+a,f]   (i >= 8, else 0)
    C = ar_params + ma_params

Flattening each (b,n) sequence to a stream of S*F elements, the conv is a
banded linear map on 256-element blocks of the stream:

    out[256C + u] = sum_lag C[8-lag, u%8] * xpad[256C + 128 + (u - 8*lag)]

i.e. three 128-contraction matmuls per 256-wide output block, with small
banded weight matrices built from C on the host.  Data-parallel over 8 cores
(100 sequences each); no cross-device communication.

Mode "pe" (default) per-core pipeline:
  - host: cast x to fp16 and build a (128, L+128) sliding-window view of the
    padded stream (partition p owns span [p*L, p*L+L) plus a 128-elem left
    halo; L = 25600).
  - plain big-descriptor DMA loads (6.4KB/partition chunks, ~350 GB/s);
    NO DMA-xbar transpose (the xbar serializes against every other DMA on
    the core and caps at ~240 GB/s, measured).
  - PE transpose-mode matmuls (~75ns/tile) build the X^T tiles on-chip in
    f16 PSUM quads; DVE/ACT copy them to SBUF.
  - conv: per 256-wide output group g, 3 matmuls contract the transposed
    tiles T_{2g}, T_{2g+1}, T_{2g+2} against banded weights [W0|Wm1]
    (the first matmul is N=256 with zero-padded weights so its start=True
    clears every PSUM has_written bit the group touches).
  - PSUM (fp32) -> SBUF copies cast to fp16 and stage 10 groups; output rows
    are span-major, so the store DMA writes 5.1KB-contiguous DRAM rows and
    overlaps the input loads (both plain DMA -> no xbar serialization).
  - host: reassemble spans, cast fp16->fp32, zero the first 8 stations of
    every sequence (conv warm-up region; also absorbs the cross-sequence
    contamination in the flat stream).

End-to-end absmax-relative error vs the fp32 reference: ~6e-4 (fp16 input /
weight / output quantization; accumulation is fp32 in PSUM).

Other modes kept for reference/fallback: "fp16" (DMA-xbar transpose input
path, phase-separated from the output stream; ~64us), "hybrid" (xbar + PE
transpose split; ~75us), "bf16_split" (x and C each split into bf16 hi+lo
parts -> ~8.5e-6 rel err at ~146us, fp32-grade accuracy fallback).
"""

import numpy as np
import ml_dtypes

BF16 = ml_dtypes.bfloat16

MODE = "pe"                             # "pe" | "hybrid" | "fp16" | "bf16_split"

B, N, S, F = 4, 200, 4096, 8
K = 8
NCORES = 8
P = 128
SEQ_PER_CORE = B * N // NCORES          # 100
STREAM = SEQ_PER_CORE * S * F           # 3,276,800 elements per core
NBLK = STREAM // P                      # 25,600 blocks of 128

# fp16-mode tiling
CB = 5120                               # 128-blocks per chunk
OT_BANKS = 5                            # PSUM banks staged per output DMA

# bf16_split-mode tiling
SP_CB = 5120
SP_GROUP = 4
SP_OT_GROUPS = 5

_compiled = {}


# --------------------------------------------------------------------------
# fp16 mode
# --------------------------------------------------------------------------

def _make_nc_fp16(nblk, cb, ot_banks, n_cores):
    import concourse.mybir as mybir
    import concourse.tile as tile
    from concourse import bacc

    chunks = nblk // cb
    assert chunks * cb == nblk
    tw = cb + P                         # transposed cols per chunk (halo incl.)
    tw2 = tw // 2
    ncoarse = nblk // 2                 # 256-elem output blocks per core
    subtiles_per_chunk = cb // 256      # psum half-bank groups of 128 coarse
    banks_per_chunk = subtiles_per_chunk // 2
    otiles_per_chunk = banks_per_chunk // ot_banks
    assert otiles_per_chunk * ot_banks == banks_per_chunk
    ot_cols = ot_banks * 512            # output cols per staging tile

    nc = bacc.Bacc(
        "TRN2", target_bir_lowering=False, debug=False, num_devices=n_cores
    )
    f16 = mybir.dt.float16
    f32 = mybir.dt.float32

    # chunked + parity-deinterleaved input: x_d[c, j, :] rows are the chunk's
    # even 128-blocks then its odd 128-blocks (host lays this out)
    x_d = nc.dram_tensor("x16", [chunks, tw, P], f16, kind="ExternalInput")
    # weights: [W0 (256 cols, zero-padded) | Wm1 (64 cols)], stored
    # TRANSPOSED on host so the load can use the xbar-transpose path (keeps
    # phase 1 free of DMA-mode transitions)
    w_d = nc.dram_tensor("wts", [320, P], f16, kind="ExternalInput")
    y_d = nc.dram_tensor("y", [ncoarse, 256], f16, kind="ExternalOutput")

    def _ins(x):
        return getattr(x, "ins", x)

    with tile.TileContext(nc) as tc:
        from concourse.tile import add_dep_helper
        with tc.tile_pool(name="wpool", bufs=1) as wpool, \
             tc.tile_pool(name="xpool", bufs=chunks) as xpool, \
             tc.tile_pool(name="psum", bufs=8, space="PSUM") as psum, \
             tc.tile_pool(name="opool", bufs=chunks * otiles_per_chunk) as opool:
            W = wpool.tile([P, 320], f16)
            nc.sync.dma_start(out=W[:], in_=w_d[:], transpose=True)
            # Phase 1: all xbar transposes (SP ring), with PE matmuls and
            # PSUM->SBUF copies overlapping as chunks land.  Phase 2: output
            # DMAs, explicitly held until the LAST transpose completes -- the
            # HW xbar-mode bug forces Tile to serialize any transpose/copy
            # DMA pair, so interleaving them thrashes; one transition is free.
            tr_insts = []
            out_calls = []
            copy_flip = 0
            for c in range(chunks):
                xt = xpool.tile([P, tw], f16, tag="xt")
                tr = nc.sync.dma_start(out=xt[:], in_=x_d[c], transpose=True)
                tr_insts.append(_ins(tr))
                for ot in range(otiles_per_chunk):
                    otile = opool.tile([P, ot_cols], f16)
                    for g in range(ot_banks):
                        pt = psum.tile([P, 512], f32)
                        for half in range(2):
                            i = (ot * ot_banks + g) * 2 + half
                            A = i * P
                            o0 = half * 256
                            # S0 = odd blocks, S1/Sm1 = even blocks
                            s0 = xt[:, tw2 + A: tw2 + A + P]
                            s1 = xt[:, A + 1: A + 1 + P]
                            sm1 = xt[:, A: A + P]
                            nc.tensor.matmul(pt[:, o0: o0 + 256], s0,
                                             W[:, 0:256],
                                             start=True, stop=False)
                            nc.tensor.matmul(pt[:, o0 + 128: o0 + 256], s1,
                                             W[:, 0:128],
                                             start=False, stop=False)
                            nc.tensor.matmul(pt[:, o0: o0 + 64], sm1,
                                             W[:, 256:320],
                                             start=False, stop=True)
                        odst = otile[:, g * 512:(g + 1) * 512]
                        if copy_flip % 2 == 0:
                            nc.vector.tensor_copy(odst, pt[:])
                        else:
                            nc.scalar.copy(odst, pt[:])
                        copy_flip += 1
                    base = (c * banks_per_chunk + ot * ot_banks) * 256
                    out = nc.scalar.dma_start(
                        out=y_d[base: base + ot_banks * 256, :].rearrange(
                            "(m p) u -> p m u", p=P
                        ),
                        in_=otile[:].rearrange("p (m u) -> p m u", u=256),
                    )
                    out_calls.append(_ins(out))
            for o in out_calls:
                add_dep_helper(o, tr_insts[-1],
                               reason="hold output DMAs until last transpose")
    nc.compile()
    return nc


def _build_wts_fp16(Cmat, transposed=True):
    """[W0(256, zero-padded) | Wm1(64)] from C (8x8 fp32), in fp16.

    out[256C+u] = sum_lag C[8-lag, u%8] * xpad[256C+128 + (u-8*lag)]
      S0[v]  = xpad[256C+128+v]  -> W0[v, v+8lag]            (u = v+8lag)
      S1[v]  = xpad[256C+256+v]  -> W0[v, v+8lag] cols <128  (u = 128+v+8lag)
      Sm1[v] = xpad[256C+v]      -> Wm1[v, v-128+8lag]       (u = v-128+8lag)
    """
    C16 = Cmat.astype(np.float16).astype(np.float32)
    W0 = np.zeros((P, 256), np.float32)
    Wm1 = np.zeros((P, 64), np.float32)
    for v in range(P):
        f = v % 8
        for lag in range(1, 9):
            u = v + 8 * lag
            if u < 256:
                W0[v, u] = C16[8 - lag, f]
            um = v - 128 + 8 * lag
            if 0 <= um < 64:
                Wm1[v, um] = C16[8 - lag, f]
    W = np.concatenate([W0, Wm1], axis=1)
    if transposed:
        W = np.ascontiguousarray(W.T)
    return W.astype(np.float16)


def _prep_in_maps_fp16(x, ar_params, ma_params, n_cores, stream, nblk, cb):
    chunks = nblk // cb
    tw = cb + P
    padded = nblk + P
    Cmat = np.asarray(ar_params, np.float32) + np.asarray(ma_params, np.float32)
    wts = _build_wts_fp16(Cmat)
    xf = np.ascontiguousarray(np.asarray(x, dtype=np.float32)).reshape(
        n_cores, stream
    )
    pad = np.zeros((n_cores, padded, P), np.float16)
    pad[:, 1:1 + nblk, :] = xf.astype(np.float16).reshape(n_cores, nblk, P)
    # per-chunk parity de-interleave: even blocks then odd blocks
    perm = np.concatenate([np.arange(0, tw, 2), np.arange(1, tw, 2)])
    xd = np.empty((n_cores, chunks, tw, P), np.float16)
    for c in range(chunks):
        xd[:, c] = pad[:, c * cb: c * cb + tw, :][:, perm, :]
    return [
        {"x16": xd[core], "wts": wts} for core in range(n_cores)
    ]


# --------------------------------------------------------------------------
# bf16_split mode (fp32-grade fallback)
# --------------------------------------------------------------------------

def _make_nc_split(nblk, cb, group, ot_groups, n_cores):
    import concourse.mybir as mybir
    import concourse.tile as tile
    from concourse import bacc

    chunks = nblk // cb
    assert chunks * cb == nblk
    tw = cb + P
    tiles_per_chunk = cb // P
    groups_per_chunk = tiles_per_chunk // group
    otiles_per_chunk = groups_per_chunk // ot_groups
    assert otiles_per_chunk * ot_groups == groups_per_chunk
    ot_cols = ot_groups * group * P

    nc = bacc.Bacc(
        "TRN2", target_bir_lowering=False, debug=False, num_devices=n_cores
    )
    bf = mybir.dt.bfloat16
    f32 = mybir.dt.float32

    xh_d = nc.dram_tensor("xh", [nblk + P, P], bf, kind="ExternalInput")
    xl_d = nc.dram_tensor("xl", [nblk + P, P], bf, kind="ExternalInput")
    w_d = nc.dram_tensor("wts", [4, P, P], bf, kind="ExternalInput")
    y_d = nc.dram_tensor("y", [nblk, P], f32, kind="ExternalOutput")

    with tile.TileContext(nc) as tc:
        with tc.tile_pool(name="wpool", bufs=1) as wpool, \
             tc.tile_pool(name="xpool", bufs=2) as xpool, \
             tc.tile_pool(name="psum", bufs=6, space="PSUM") as psum, \
             tc.tile_pool(name="opool", bufs=2) as opool:
            W = wpool.tile([P, 4, P], bf)
            for j in range(4):
                nc.sync.dma_start(out=W[:, j, :], in_=w_d[j])
            for c in range(chunks):
                xh_t = xpool.tile([P, tw], bf, tag="xh_t")
                xl_t = xpool.tile([P, tw], bf, tag="xl_t")
                nc.sync.dma_start(
                    out=xh_t[:], in_=xh_d[c * cb: c * cb + tw, :],
                    transpose=True
                )
                nc.sync.dma_start(
                    out=xl_t[:], in_=xl_d[c * cb: c * cb + tw, :],
                    transpose=True
                )
                for ot in range(otiles_per_chunk):
                    otile = opool.tile([P, ot_cols], f16)
                    for g in range(ot_groups):
                        pt = psum.tile([P, group * P], f32)
                        for k in range(group):
                            t = (ot * ot_groups + g) * group + k
                            s = t * P
                            o = pt[:, k * P:(k + 1) * P]
                            nc.tensor.matmul(o, xh_t[:, s + 1: s + 1 + P],
                                             W[:, 0, :], start=True, stop=False)
                            nc.tensor.matmul(o, xh_t[:, s + 1: s + 1 + P],
                                             W[:, 2, :], start=False, stop=False)
                            nc.tensor.matmul(o, xh_t[:, s: s + P],
                                             W[:, 1, :], start=False, stop=False)
                            nc.tensor.matmul(o, xh_t[:, s: s + P],
                                             W[:, 3, :], start=False, stop=False)
                            nc.tensor.matmul(o, xl_t[:, s + 1: s + 1 + P],
                                             W[:, 0, :], start=False, stop=False)
                            nc.tensor.matmul(o, xl_t[:, s: s + P],
                                             W[:, 1, :], start=False, stop=True)
                        odst = otile[:, g * group * P:(g + 1) * group * P]
                        if g % 2 == 0:
                            nc.vector.tensor_copy(odst, pt[:])
                        else:
                            nc.scalar.copy(odst, pt[:])
                    base = c * cb + ot * ot_cols
                    nc.scalar.dma_start(
                        out=y_d[base: base + ot_cols, :].rearrange(
                            "(k p) u -> p k u", p=P
                        ),
                        in_=otile[:].rearrange("p (k u) -> p k u", u=P),
                    )
    nc.compile()
    return nc


def _mk_AB(Cm):
    A = np.zeros((P, P), np.float32)
    Bm = np.zeros((P, P), np.float32)
    for u in range(P):
        f = u % 8
        for lag in range(1, 9):
            coef = Cm[8 - lag, f]
            v = u - 8 * lag
            if v >= 0:
                A[v, u] = coef
            else:
                Bm[v + P, u] = coef
    return A, Bm


def _build_wts_split(Cmat):
    Chi = Cmat.astype(BF16).astype(np.float32)
    Clo = (Cmat - Chi).astype(BF16).astype(np.float32)
    Ahi, Bhi = _mk_AB(Chi)
    Alo, Blo = _mk_AB(Clo)
    return np.stack([Ahi, Bhi, Alo, Blo]).astype(BF16)


def _prep_in_maps_split(x, ar_params, ma_params, n_cores, stream, nblk):
    padded = nblk + P
    Cmat = np.asarray(ar_params, np.float32) + np.asarray(ma_params, np.float32)
    wts = _build_wts_split(Cmat)
    xf = np.ascontiguousarray(np.asarray(x, dtype=np.float32)).reshape(
        n_cores, stream
    )
    xh = xf.astype(BF16)
    xl = (xf - xh.astype(np.float32)).astype(BF16)
    ph = np.zeros((n_cores, padded * P), BF16)
    ph[:, P:P + stream] = xh
    pl = np.zeros((n_cores, padded * P), BF16)
    pl[:, P:P + stream] = xl
    return [
        {
            "xh": ph[c].reshape(padded, P),
            "xl": pl[c].reshape(padded, P),
            "wts": wts,
        }
        for c in range(n_cores)
    ]


# --------------------------------------------------------------------------
# pe mode: no DMA-xbar at all.  Plain big-descriptor loads (overlap the
# output stream freely), PE transpose-mode matmuls build the X^T tiles
# on-chip, and the span-major layout makes output rows ~10KB contiguous.
# --------------------------------------------------------------------------

def _make_nc_pe(L, load_cols, g_stage, n_cores):
    import concourse.mybir as mybir
    import concourse.tile as tile
    from concourse import bacc

    NJ = L // P + 1                      # 128-col transpose tiles (incl halo)
    NG = L // 256                        # 256-elem output groups per partition
    assert (NJ - 1) % (load_cols // P) == 0
    load_plan = [load_cols // P] * ((NJ - 1) // (load_cols // P))
    assert NG % g_stage == 0
    notiles = NG // g_stage

    nc = bacc.Bacc(
        "TRN2", target_bir_lowering=False, debug=False, num_devices=n_cores
    )
    f16 = mybir.dt.float16
    f32 = mybir.dt.float32

    x_d = nc.dram_tensor("xin", [P, L + P], f16, kind="ExternalInput")
    w_d = nc.dram_tensor("wts", [P, 320], f16, kind="ExternalInput")
    id_d = nc.dram_tensor("ident", [P, P], f16, kind="ExternalInput")
    y_d = nc.dram_tensor("y", [P, L], f16, kind="ExternalOutput")

    with tile.TileContext(nc) as tc:
        with tc.tile_pool(name="wpool", bufs=1) as wpool, \
             tc.tile_pool(name="xpool", bufs=len(load_plan) + 1) as xpool, \
             tc.tile_pool(name="tq", bufs=4) as tqpool, \
             tc.tile_pool(name="pst", bufs=3, space="PSUM") as pst, \
             tc.tile_pool(name="pso", bufs=4, space="PSUM") as pso, \
             tc.tile_pool(name="opool", bufs=2) as opool:
            W = wpool.tile([P, 320], f16, tag="w")
            ident = wpool.tile([P, P], f16, tag="ident")
            nc.sync.dma_start(out=W[:], in_=w_d[:])
            nc.sync.dma_start(out=ident[:], in_=id_d[:])
            xts = []          # (tile, j_base, n_j)
            jb = 0
            for nj in load_plan:
                xt = xpool.tile([P, nj * P], f16, tag="xin")
                nc.sync.dma_start(
                    out=xt[:], in_=x_d[:, jb * P:(jb + nj) * P]
                )
                xts.append((xt, jb, nj))
                jb += nj
            xhalo = xpool.tile([P, P], f16, tag="xhalo")
            nc.sync.dma_start(out=xhalo[:], in_=x_d[:, L:])

            def src_of(j):
                if j == NJ - 1:
                    return xhalo[:, 0:P]
                for xt, jb2, nj in xts:
                    if jb2 <= j < jb2 + nj:
                        return xt[:, (j - jb2) * P:(j - jb2 + 1) * P]
                raise AssertionError(j)

            # T_j (transposed tiles) are built in quads: 4 PE transposes into
            # one f16 PSUM bank, one copy out to SBUF.
            tq_tiles = {}                # quad index -> sbuf tile
            def t_of(j):
                q, off = j // 4, (j % 4) * P
                return tq_tiles[q][:, off: off + P]

            nquads = (NJ + 3) // 4
            copy_flip = 0
            g_next = 0
            otile = None
            for q in range(nquads):
                ptile = pst.tile([P, 512], f16)
                j_hi = min(4 * q + 4, NJ)
                for j in range(4 * q, j_hi):
                    nc.tensor.transpose(
                        ptile[:, (j % 4) * P:(j % 4 + 1) * P], src_of(j),
                        ident[:]
                    )
                tqt = tqpool.tile([P, 512], f16, tag="tq")
                if q % 2 == 0:
                    nc.vector.tensor_copy(tqt[:], ptile[:])
                else:
                    nc.scalar.copy(tqt[:], ptile[:])
                tq_tiles[q] = tqt
                # emit conv groups whose inputs are now all transposed
                while g_next < NG and 2 * g_next + 2 < j_hi:
                    g = g_next
                    if g % 2 == 0:
                        po = pso.tile([P, 512], f32)
                    o0 = (g % 2) * 256
                    nc.tensor.matmul(po[:, o0: o0 + 256], t_of(2 * g + 1),
                                     W[:, 0:256], start=True, stop=False)
                    nc.tensor.matmul(po[:, o0 + 128: o0 + 256], t_of(2 * g + 2),
                                     W[:, 0:128], start=False, stop=False)
                    nc.tensor.matmul(po[:, o0: o0 + 64], t_of(2 * g),
                                     W[:, 256:320], start=False, stop=True)
                    if g % 2 == 1:
                        if g // 2 % (g_stage // 2) == 0:
                            otile = opool.tile([P, g_stage * 256], f16,
                                               tag="ot")
                        oc = (g // 2 % (g_stage // 2)) * 512
                        odst = otile[:, oc: oc + 512]
                        if copy_flip % 2 == 0:
                            nc.vector.tensor_copy(odst, po[:])
                        else:
                            nc.scalar.copy(odst, po[:])
                        copy_flip += 1
                        if (g + 1) % g_stage == 0:
                            o_idx = g // g_stage
                            nc.scalar.dma_start(
                                out=y_d[:, o_idx * g_stage * 256:
                                        (o_idx + 1) * g_stage * 256],
                                in_=otile[:],
                            )
                    g_next += 1
    nc.compile()
    return nc


def _prep_in_maps_pe(x, ar_params, ma_params, n_cores, stream, L):
    Cmat = np.asarray(ar_params, np.float32) + np.asarray(ma_params, np.float32)
    wts = _build_wts_fp16(Cmat, transposed=False)
    xf = np.ascontiguousarray(np.asarray(x, dtype=np.float32)).reshape(
        n_cores, stream
    )
    xpad = np.zeros((n_cores, P + stream), np.float16)
    xpad[:, P:] = xf.astype(np.float16)
    ident = np.eye(P, dtype=np.float16)
    maps = []
    for c in range(n_cores):
        win = np.lib.stride_tricks.as_strided(
            xpad[c], (P, L + P), (L * 2, 2)
        )
        maps.append({
            "xin": np.ascontiguousarray(win),
            "wts": wts,
            "ident": ident,
        })
    return maps


# --------------------------------------------------------------------------
# hybrid mode: stream split in two.  Part A goes through the DMA-xbar
# transpose path in an exclusive phase (the xbar serializes against every
# other DMA, so nothing else moves while it runs -- but the PE computes A's
# convolutions underneath it).  Part B uses plain big-descriptor loads +
# PE transpose-mode, and all output stores run in phase B where they overlap
# the B loads.  W / identity are ALSO loaded via the xbar so phase A contains
# no DMA mode transitions at all.
# --------------------------------------------------------------------------

def _make_nc_hybrid(nblkA, cbA, ot_banksA, L_B, load_colsB, g_stageB, n_cores):
    import concourse.mybir as mybir
    import concourse.tile as tile
    from concourse import bacc
    from concourse.tile import add_dep_helper

    # ---- A-side geometry (xbar path, fp16-mode structure)
    chunksA = nblkA // cbA
    assert chunksA * cbA == nblkA
    twA = cbA + P
    twA2 = twA // 2
    ncoarseA = nblkA // 2
    subtilesA = cbA // 256
    banksA = subtilesA // 2
    otilesA = banksA // ot_banksA
    assert otilesA * ot_banksA == banksA
    ot_colsA = ot_banksA * 512

    # ---- B-side geometry (pe path)
    NJ = L_B // P + 1
    NG = L_B // 256
    jgrp = load_colsB // P
    assert (NJ - 1) % jgrp == 0
    nloadsB = (NJ - 1) // jgrp
    assert NG % g_stageB == 0

    nc = bacc.Bacc(
        "TRN2", target_bir_lowering=False, debug=False, num_devices=n_cores
    )
    f16 = mybir.dt.float16
    f32 = mybir.dt.float32

    xA_d = nc.dram_tensor("xA", [chunksA, twA, P], f16, kind="ExternalInput")
    xB_d = nc.dram_tensor("xB", [P, L_B + P], f16, kind="ExternalInput")
    w_d = nc.dram_tensor("wts", [320, P], f16, kind="ExternalInput")
    id_d = nc.dram_tensor("ident", [P, P], f16, kind="ExternalInput")
    yA_d = nc.dram_tensor("yA", [ncoarseA, 256], f16, kind="ExternalOutput")
    yB_d = nc.dram_tensor("yB", [P, L_B], f16, kind="ExternalOutput")

    def _ins(x):
        return getattr(x, "ins", x)

    plain_dmas = []
    early_loads = []
    with tile.TileContext(nc) as tc:
        with tc.tile_pool(name="wpool", bufs=1) as wpool, \
             tc.tile_pool(name="xpoolA", bufs=chunksA) as xpoolA, \
             tc.tile_pool(name="xpoolB", bufs=nloadsB + 1) as xpoolB, \
             tc.tile_pool(name="tq", bufs=4) as tqpool, \
             tc.tile_pool(name="psA", bufs=3, space="PSUM") as psA, \
             tc.tile_pool(name="pst", bufs=2, space="PSUM") as pst, \
             tc.tile_pool(name="psB", bufs=3, space="PSUM") as psB, \
             tc.tile_pool(name="opoolA", bufs=otilesA * chunksA) as opoolA, \
             tc.tile_pool(name="opoolB", bufs=NG // g_stageB) as opoolB:
            W = wpool.tile([P, 320], f16, tag="w")
            ident = wpool.tile([P, P], f16, tag="ident")

            # phase 0: ALL plain input loads (B spans), before any xbar use
            xts = []
            for gl in range(nloadsB):
                xbt = xpoolB.tile([P, load_colsB], f16, tag="xinB")
                ld = nc.sync.dma_start(
                    out=xbt[:],
                    in_=xB_d[:, gl * load_colsB:(gl + 1) * load_colsB],
                )
                early_loads.append(_ins(ld))
                xts.append(xbt)
            xhalo = xpoolB.tile([P, P], f16, tag="xhaloB")
            ldh = nc.sync.dma_start(out=xhalo[:], in_=xB_d[:, L_B:])
            early_loads.append(_ins(ldh))

            # phase X: xbar transposes (W, ident, A chunks); PE does B work
            wtr = nc.sync.dma_start(out=W[:], in_=w_d[:], transpose=True)
            itr = nc.sync.dma_start(out=ident[:], in_=id_d[:], transpose=True)
            tr_insts = [_ins(wtr), _ins(itr)]
            xtAs = []
            for c in range(chunksA):
                xtA = xpoolA.tile([P, twA], f16, tag="xtA")
                tr = nc.sync.dma_start(out=xtA[:], in_=xA_d[c], transpose=True)
                tr_insts.append(_ins(tr))
                xtAs.append(xtA)
            # xbar only after the plain loads have fully drained
            for t in tr_insts:
                for el in early_loads:
                    add_dep_helper(t, el, sync=True,
                                   reason="xbar waits for plain input loads")

            copy_flip = 0

            # ---- B section: PE transposes + convs (data from phase 0)
            tq_tiles = {}

            def t_of(j):
                q, off = j // 4, (j % 4) * P
                return tq_tiles[q][:, off: off + P]

            def src_of(j):
                if j == NJ - 1:
                    return xhalo[:, 0:P]
                return xts[j // jgrp][:, (j % jgrp) * P:(j % jgrp + 1) * P]

            nquads = (NJ + 3) // 4
            g_next = 0
            otile = None
            for q in range(nquads):
                ptile = pst.tile([P, 512], f16)
                j_hi = min(4 * q + 4, NJ)
                for j in range(4 * q, j_hi):
                    nc.tensor.transpose(
                        ptile[:, (j % 4) * P:(j % 4 + 1) * P], src_of(j),
                        ident[:]
                    )
                tqt = tqpool.tile([P, 512], f16, tag="tq")
                if q % 2 == 0:
                    nc.vector.tensor_copy(tqt[:], ptile[:])
                else:
                    nc.scalar.copy(tqt[:], ptile[:])
                tq_tiles[q] = tqt
                while g_next < NG and 2 * g_next + 2 < j_hi:
                    g = g_next
                    if g % 2 == 0:
                        po = psB.tile([P, 512], f32)
                    o0 = (g % 2) * 256
                    nc.tensor.matmul(po[:, o0: o0 + 256], t_of(2 * g + 1),
                                     W[:, 0:256], start=True, stop=False)
                    nc.tensor.matmul(po[:, o0 + 128: o0 + 256], t_of(2 * g + 2),
                                     W[:, 0:128], start=False, stop=False)
                    nc.tensor.matmul(po[:, o0: o0 + 64], t_of(2 * g),
                                     W[:, 256:320], start=False, stop=True)
                    if g % 2 == 1:
                        if g // 2 % (g_stageB // 2) == 0:
                            otile = opoolB.tile([P, g_stageB * 256], f16,
                                                tag="otB")
                        oc = (g // 2 % (g_stageB // 2)) * 512
                        odst = otile[:, oc: oc + 512]
                        if copy_flip % 2 == 0:
                            nc.vector.tensor_copy(odst, po[:])
                        else:
                            nc.scalar.copy(odst, po[:])
                        copy_flip += 1
                        if (g + 1) % g_stageB == 0:
                            o_idx = g // g_stageB
                            outb = nc.scalar.dma_start(
                                out=yB_d[:, o_idx * g_stageB * 256:
                                         (o_idx + 1) * g_stageB * 256],
                                in_=otile[:],
                            )
                            plain_dmas.append(_ins(outb))
                    g_next += 1

            # ---- A section: convs on xbar-transposed tiles
            for c in range(chunksA):
                xtA = xtAs[c]
                for ot in range(otilesA):
                    otileA = opoolA.tile([P, ot_colsA], f16, tag="otA")
                    for g in range(ot_banksA):
                        pt = psA.tile([P, 512], f32)
                        for half in range(2):
                            i = (ot * ot_banksA + g) * 2 + half
                            A0 = i * P
                            o0 = half * 256
                            s0 = xtA[:, twA2 + A0: twA2 + A0 + P]
                            s1 = xtA[:, A0 + 1: A0 + 1 + P]
                            sm1 = xtA[:, A0: A0 + P]
                            nc.tensor.matmul(pt[:, o0: o0 + 256], s0,
                                             W[:, 0:256],
                                             start=True, stop=False)
                            nc.tensor.matmul(pt[:, o0 + 128: o0 + 256], s1,
                                             W[:, 0:128],
                                             start=False, stop=False)
                            nc.tensor.matmul(pt[:, o0: o0 + 64], sm1,
                                             W[:, 256:320],
                                             start=False, stop=True)
                        odst = otileA[:, g * 512:(g + 1) * 512]
                        if copy_flip % 2 == 0:
                            nc.vector.tensor_copy(odst, pt[:])
                        else:
                            nc.scalar.copy(odst, pt[:])
                        copy_flip += 1
                    base = (c * banksA + ot * ot_banksA) * 256
                    outa = nc.scalar.dma_start(
                        out=yA_d[base: base + ot_banksA * 256, :].rearrange(
                            "(m p) u -> p m u", p=P
                        ),
                        in_=otileA[:].rearrange("p (m u) -> p m u", u=256),
                    )
                    plain_dmas.append(_ins(outa))

            for pd in plain_dmas:
                add_dep_helper(pd, tr_insts[-1],
                               reason="hold plain DMAs until last xbar transpose")
    nc.compile()
    return nc


def _prep_in_maps_hybrid(x, ar_params, ma_params, n_cores, stream,
                         nblkA, cbA, L_B):
    streamA = nblkA * P
    chunksA = nblkA // cbA
    twA = cbA + P
    paddedA = nblkA + P
    Cmat = np.asarray(ar_params, np.float32) + np.asarray(ma_params, np.float32)
    wts = _build_wts_fp16(Cmat, transposed=True)
    ident = np.ascontiguousarray(np.eye(P, dtype=np.float16))
    xf = np.ascontiguousarray(np.asarray(x, dtype=np.float32)).reshape(
        n_cores, stream
    )
    x16 = xf.astype(np.float16)
    # full padded stream (front 128 zeros) once per core
    xpadF = np.zeros((n_cores, P + stream), np.float16)
    xpadF[:, P:] = x16
    # A: chunked + parity-deinterleaved view of padded blocks [0, nblkA+P)
    padA = np.zeros((n_cores, paddedA, P), np.float16)
    padA.reshape(n_cores, -1)[:, :streamA + P] = xpadF[:, :streamA + P]
    perm = np.concatenate([np.arange(0, twA, 2), np.arange(1, twA, 2)])
    xA = np.empty((n_cores, chunksA, twA, P), np.float16)
    for c in range(chunksA):
        xA[:, c] = padA[:, c * cbA: c * cbA + twA, :][:, perm, :]
    maps = []
    for core in range(n_cores):
        winB = np.lib.stride_tricks.as_strided(
            xpadF[core, streamA:], (P, L_B + P), (L_B * 2, 2)
        )
        maps.append({
            "xA": xA[core],
            "xB": np.ascontiguousarray(winB),
            "wts": wts,
            "ident": ident,
        })
    return maps


# --------------------------------------------------------------------------
# driver
# --------------------------------------------------------------------------

HY_NBLKA = 12800
HY_CBA = 2560
HY_OTBA = 5
HY_LB = 12800
HY_LOADB = 3200
HY_GSTB = 10


def _get_nc(mode=MODE, **kw):
    if mode == "hybrid":
        key = ("hybrid", HY_NBLKA, HY_CBA, HY_OTBA, HY_LB, HY_LOADB, HY_GSTB,
               kw.get("n_cores", NCORES))
        if key not in _compiled:
            _compiled[key] = _make_nc_hybrid(*key[1:])
        return _compiled[key]
    if mode == "pe":
        key = ("pe", kw.get("L", STREAM // P), kw.get("load_cols", 3200),
               kw.get("g_stage", 20), kw.get("n_cores", NCORES))
        if key not in _compiled:
            _compiled[key] = _make_nc_pe(*key[1:])
        return _compiled[key]
    if mode == "fp16":
        key = ("fp16", kw.get("nblk", NBLK), kw.get("cb", CB),
               kw.get("ot_banks", OT_BANKS), kw.get("n_cores", NCORES))
        if key not in _compiled:
            _compiled[key] = _make_nc_fp16(*key[1:])
    else:
        key = ("split", kw.get("nblk", NBLK), kw.get("cb", SP_CB),
               kw.get("group", SP_GROUP), kw.get("ot_groups", SP_OT_GROUPS),
               kw.get("n_cores", NCORES))
        if key not in _compiled:
            _compiled[key] = _make_nc_split(*key[1:])
    return _compiled[key]


def _ensure_hook_shim():
    """run_bass_kernel_spmd(trace=True) imports antenv.axon_hooks, which the
    agent image may lack; also BASS_TRACE in the env triggers that path.
    Install a null shim so the import never crashes the kernel."""
    import sys
    import types
    try:
        import antenv.axon_hooks  # noqa: F401
    except Exception:
        mod = types.ModuleType("antenv.axon_hooks")
        mod.get_axon_ntff_profile_hook = lambda: None
        mod.set_axon_ntff_profile_hook = lambda h: None
        sys.modules["antenv.axon_hooks"] = mod


def _run(x, ar_params, ma_params, trace=False, mode=MODE, **run_kwargs):
    _ensure_hook_shim()
    from concourse.bass_utils import run_bass_kernel_spmd

    nc = _get_nc(mode)
    if mode == "hybrid":
        in_maps = _prep_in_maps_hybrid(x, ar_params, ma_params, NCORES, STREAM,
                                       HY_NBLKA, HY_CBA, HY_LB)
    elif mode == "pe":
        in_maps = _prep_in_maps_pe(x, ar_params, ma_params, NCORES, STREAM,
                                   STREAM // P)
    elif mode == "fp16":
        in_maps = _prep_in_maps_fp16(x, ar_params, ma_params, NCORES, STREAM,
                                     NBLK, CB)
    else:
        in_maps = _prep_in_maps_split(x, ar_params, ma_params, NCORES, STREAM,
                                      NBLK)
    res = run_bass_kernel_spmd(
        nc, in_maps, core_ids=list(range(NCORES)), trace=trace, **run_kwargs
    )
    if mode == "hybrid":
        out = np.empty((NCORES, STREAM), np.float32)
        sa = HY_NBLKA * P
        for c in range(NCORES):
            out[c, :sa] = np.asarray(
                res.results[c]["yA"], dtype=np.float32).reshape(-1)
            out[c, sa:] = np.asarray(
                res.results[c]["yB"], dtype=np.float32).reshape(-1)
    else:
        out = np.stack(
            [np.asarray(res.results[c]["y"], dtype=np.float32)
             for c in range(NCORES)]
        )
    out = out.reshape(B, N, S, F)
    out[:, :, :K, :] = 0.0
    return out, res


def kernel(x, ar_params, ma_params):
    out, _ = _run(x, ar_params, ma_params)
    return out
